# revision 1
# baseline (speedup 1.0000x reference)
"""TRN2 Bass kernel for nn_Blur: upfirdn2d(pad=(2,1)) with a separable 4x4
binomial FIR, x shape (8, 256, 256, 256) f32, depthwise per (n, c) plane.

Strategy
--------
Batch-parallel across the 8 NeuronCores (core i gets x[i]).

The FIR is separable: out = T_H^T @ X @ T_W per (c) plane, where T_H/T_W are
256x256 banded Toeplitz matrices (band [k1[0..3]] on diagonals -1..+2, zero
boundary = the reference's zero padding).

Both passes run on the TensorEngine with the *data* as the stationary
operand (lhsT) and the Toeplitz as the moving operand (rhs):

  pass1:  Y^T = X^T @ T_H      (lhsT = X tile   [h_in=128, w=128],
                                rhs  = T_H blk  [h_in=128, h'=256])
  pass2:  Z   = Y  @ T_W       (lhsT = Y^T tile [w_in=128, h'=128],
                                rhs  = T_W blk  [w_in=128, w'=256])

so no transposes are ever needed: pass1 naturally yields Y^T, pass2
naturally yields Z in output layout.

Precision: pass1 data is split on the host into bf16 hi + lo halves
(x = hi + lo exactly to ~2^-18), and the Toeplitz entries are exact in
bf16, so pass1 is fp32-accurate while running at bf16 matmul speed.
Pass2 runs in PASS2_DTYPE (float32 = exact, 4 cyc/row; float32r = ~1e-4,
1 cyc/row).
"""
import numpy as np
import ml_dtypes

import concourse.bacc as bacc
import concourse.mybir as mybir
from concourse.tile import TileContext
from concourse.bass_utils import run_bass_kernel_spmd

N, C, H, W = 8, 256, 256, 256
P = 128          # partition size
NCORES = 8
KSIZE = 4        # FIR tap count
# band: T[i, i+d] = k1[d+1], d in {-1, 0, 1, 2}
BAND_LO, BAND_HI = -1, 2
# nonzero column ranges of the two 128-row Toeplitz blocks
BLK_COLS = [(0, P + BAND_HI), (P + BAND_LO, 2 * P)]   # [0,130), [127,256)

PASS2_DTYPE = mybir.dt.float32   # float32 (exact) or float32r (fast)

_CACHE = {}


def _factor_kernel(k2: np.ndarray):
    """Rank-1 factorization k2 = kh (x) kw (float64)."""
    k2 = np.asarray(k2, dtype=np.float64)
    u, s, vt = np.linalg.svd(k2)
    kh = u[:, 0] * np.sqrt(s[0])
    kw = vt[0] * np.sqrt(s[0])
    if kh.sum() < 0:
        kh, kw = -kh, -kw
    return kh, kw


def _toeplitz(n: int, k1: np.ndarray, dtype) -> np.ndarray:
    """T[i, j] = k1[j - i + 1] for 0 <= j-i+1 < 4, zero elsewhere."""
    t = np.zeros((n, n), dtype=np.float64)
    for d in range(BAND_LO, BAND_HI + 1):
        i = np.arange(max(0, -d), min(n, n - d))
        t[i, i + d] = k1[d + 1]
    return t.astype(dtype)


def _build(n_ch: int):
    """Build + compile the per-core Bass program (SPMD, one core's slice)."""
    nc = bacc.Bacc("TRN2", target_bir_lowering=False)

    bf16 = mybir.dt.bfloat16
    f32 = mybir.dt.float32
    p2dt = PASS2_DTYPE

    xh = nc.declare_dram_parameter("xh", [n_ch, 2, P, W], bf16, isOutput=False)
    xl = nc.declare_dram_parameter("xl", [n_ch, 2, P, W], bf16, isOutput=False)
    th = nc.declare_dram_parameter("th", [2, P, H], bf16, isOutput=False)
    tw = nc.declare_dram_parameter("tw", [2, P, W], p2dt, isOutput=False)
    out = nc.declare_dram_parameter("out", [n_ch, 2, P, W], f32, isOutput=True)

    with TileContext(nc) as tc:
        with (tc.tile_pool(name="const", bufs=1) as cpool,
              tc.tile_pool(name="xin", bufs=8) as xpool,
              tc.tile_pool(name="mid", bufs=8) as mpool,
              tc.tile_pool(name="psy", bufs=4, space="PSUM") as pypool,
              tc.tile_pool(name="psz", bufs=4, space="PSUM") as pzpool):

            tth = [cpool.tile([P, H], bf16, name=f"tth{b}", tag=f"tth{b}")
                   for b in range(2)]
            ttw = [cpool.tile([P, W], p2dt, name=f"ttw{b}", tag=f"ttw{b}")
                   for b in range(2)]
            for b in range(2):
                nc.sync.dma_start(out=tth[b][:, :], in_=th[b])
                nc.sync.dma_start(out=ttw[b][:, :], in_=tw[b])

            for c in range(n_ch):
                # ---- load x (hi, lo) : [p, hb, w] into [128, 512] tiles
                txh = xpool.tile([P, 2 * W], bf16, name="txh", tag="txh")
                txl = xpool.tile([P, 2 * W], bf16, name="txl", tag="txl")
                nc.sync.dma_start(
                    out=txh[:, :].rearrange("p (hb w) -> p hb w", hb=2),
                    in_=xh[c].rearrange("hb p w -> p hb w"))
                nc.sync.dma_start(
                    out=txl[:, :].rearrange("p (hb w) -> p hb w", hb=2),
                    in_=xl[c].rearrange("hb p w -> p hb w"))

                # ---- pass1: Y^T[wb] = sum_hb X[hb, :, wb]^T @ TH[hb]
                tyt = mpool.tile([P, 2 * H], f32, name="tyt", tag="tyt")
                for wb in range(2):
                    py = pypool.tile([P, H], f32, name="py", tag="py")
                    first = True
                    for hb in range(2):
                        lo, hi = (0, H) if first else BLK_COLS[hb]
                        for tx in (txh, txl):
                            lhsT = tx[:, hb * W + wb * P: hb * W + wb * P + P]
                            nc.tensor.matmul(
                                py[:, lo:hi], lhsT, tth[hb][:, lo:hi],
                                start=first, stop=(hb == 1 and tx is txl))
                            if first:
                                first = False
                                lo, hi = BLK_COLS[hb]
                    nc.vector.tensor_copy(tyt[:, wb * H:(wb + 1) * H], py[:, :])

                # ---- pass2: Z[hb2] = sum_wb Y^T[wb, :, hb2]^T @ TW[wb]
                tz = mpool.tile([P, 2 * W], f32, name="tz", tag="tz")
                for hb2 in range(2):
                    pz = pzpool.tile([P, W], f32, name="pz", tag="pz")
                    for wb in range(2):
                        lo, hi = (0, W) if wb == 0 else BLK_COLS[wb]
                        lhsT = tyt[:, wb * H + hb2 * P: wb * H + hb2 * P + P]
                        if p2dt != f32:
                            lhsT = lhsT.bitcast(p2dt)
                        nc.tensor.matmul(
                            pz[:, lo:hi], lhsT, ttw[wb][:, lo:hi],
                            start=(wb == 0), stop=(wb == 1))
                    nc.scalar.copy(tz[:, hb2 * W:(hb2 + 1) * W], pz[:, :])

                # ---- store
                nc.sync.dma_start(
                    out=out[c].rearrange("hb p w -> p hb w"),
                    in_=tz[:, :].rearrange("p (hb w) -> p hb w", hb=2))

    nc.compile()
    return nc


def _get_nc(n_ch: int):
    if n_ch not in _CACHE:
        _CACHE[n_ch] = _build(n_ch)
    return _CACHE[n_ch]


def _prep_inputs(x: np.ndarray, k2: np.ndarray, n_ch: int):
    kh, kw = _factor_kernel(k2)
    th = _toeplitz(H, kh, np.float32).astype(ml_dtypes.bfloat16)
    tw = _toeplitz(W, kw, mybir.dt.np(PASS2_DTYPE))
    th = np.ascontiguousarray(th.reshape(2, P, H))
    tw = np.ascontiguousarray(tw.reshape(2, P, W))

    x32 = np.asarray(x, dtype=np.float32)
    xhi = x32.astype(ml_dtypes.bfloat16)
    xlo = (x32 - xhi.astype(np.float32)).astype(ml_dtypes.bfloat16)
    # [n, c, h, w] -> [n, c, hb, p, w]
    xhi = xhi.reshape(N, n_ch, 2, P, W)
    xlo = xlo.reshape(N, n_ch, 2, P, W)

    in_maps = []
    for i in range(NCORES):
        in_maps.append({
            "xh": np.ascontiguousarray(xhi[i]),
            "xl": np.ascontiguousarray(xlo[i]),
            "th": th,
            "tw": tw,
        })
    return in_maps


def _run(x: np.ndarray, k2: np.ndarray, trace: bool = False):
    n_ch = C
    nc = _get_nc(n_ch)
    in_maps = _prep_inputs(x, k2, n_ch)
    r = run_bass_kernel_spmd(nc, in_maps, core_ids=list(range(NCORES)),
                             trace=trace)
    outs = [r.results[i]["out"].reshape(n_ch, H, W) for i in range(NCORES)]
    return np.stack(outs, axis=0), r


def kernel(x: np.ndarray, kernel: np.ndarray) -> np.ndarray:
    out, _ = _run(x, kernel, trace=False)
    return out
